# revision 26
# baseline (speedup 1.0000x reference)
"""Bass/Tile TRN2 kernel for nn_BatchAdditiveAttention.

Math (per batch, per node n):
    f_proj      = feature @ w1.T                        # (n, 128)
    t_proj[r]   = temb[:, r] @ w2.T                     # (n, 4, 128)
    q[r]        = tanh(f_proj + t_proj[r])              # (n, 4, 128)
    score[r]    = q[r] @ m                              # (n, 4)
    beta        = softmax_r(score)                      # (n, 4)
    out         = sum_r beta[r] * temb[:, r]            # (n, 256)

Sharding: data-parallel over bs=8, one batch per NeuronCore.

v3 layout/engine strategy (evolved from the v2 diag-stationary kernel;
v2 measured 368 us, v3 measures ~270-280 us on hw):
  - temb_t [R, 2, 128, N] fp8 + feat_t [2, 128, N] bf16 feed the
    projection matmuls (d on partitions, no on-chip transposes);
    the w2 projection runs as one fp8 DoubleRow matmul per type.
  - temb_n [N, R, D] fp16 is the natural-layout operand for the
    beta-weighted output reduction via diag(exp) stationaries.
    p-major node labeling (node = 4*p + a) makes both the temb_n load
    (8 KB contiguous per partition) and the output store (2 KB) long
    DMA rows; block a's q columns are the stride-4 slice a::4.
  - The 16 diag(exp) stationaries per 512-node tile are built in four
    per-block DVE tensor_tensor ops using double-broadcast APs
    (eye[p,c] x expo[p,t] -> mega[p,t,c]), replacing v2's 16 separate
    [128,128] tensor_scalar ops that made the DVE the 100%-busy
    bottleneck engine (DVE busy dropped 100% -> ~40%).
  - softmax: one batched ACT exp per tile ([128,16] -> fp16), one
    segmented DVE reduce for the per-block sums, one DVE reciprocal.
  - The final 1/sum scaling rides the PSUM->SBUF copy, alternating
    ACT Copy-with-scale / DVE tensor_scalar per block to balance the
    two engines (both sit at ~75% busy).
  - Projections process types in pairs (r_group=2) so the two PSUM
    accumulator banks recycle earlier; with qp=3/fp=3/sc=2 the eight
    PSUM banks sustain a 5-tile software pipeline (pipe=5).

Steady state: PE ~76% busy (projection + score + diag matmuls, LDW
re-emission per matmul is the main overhead), ACT ~75% (tanh + exp +
half the output copies), DMA ~235 us active vs a ~230 us floor for the
82 MB/core of traffic. Engine floors are balanced within ~15%.

Softmax skips the max-subtraction: scores are bounded (|score| <=
||m||_1 <= 11.4 in theory, ~2 in practice), well inside fp16 exp range.

Measured dead ends (do not revisit without new evidence): col-tiled
diag matmuls via tile_position (4x32-col strips, slower end-to-end);
m-stationary score matmuls (softmax then needs a partition-compacting
transpose, and strided-partition APs are rejected by the BIR verifier);
diagonal scatter-DMA into a pre-zeroed tile (illegal partition step);
walrus --enable-ldw-opt=true (rejects InstLdweights in this IR);
building diags on GpSimd (Pool cannot read PSUM, and tensor ops there
are ~2.5x DVE cost).
"""

import os
from contextlib import ExitStack

import numpy as np
import ml_dtypes

import concourse.bass as bass
import concourse.tile as tile
from concourse import bacc, mybir

BS = 8
N_NODES = 20000
D = 256
R = 4
D2 = 128
NT = 512  # nodes per tile
PB = 128  # nodes per sub-tile (partition block)

BF16 = mybir.dt.bfloat16
FP8 = mybir.dt.float8e4
F16 = mybir.dt.float16
F32 = mybir.dt.float32
AX = mybir.AxisListType
ALU = mybir.AluOpType
ACTF = mybir.ActivationFunctionType


def _sub_blocks(nt):
    """Split a node-tile of nt nodes into partition blocks of <=128.

    p-major node labeling: node n of the tile lives at partition n//4,
    block a = n%4, so block a holds ns_a = ceil((nt-a)/4) nodes.
    """
    return [(a, (nt - a + 3) // 4) for a in range(min(4, nt))]


DEFAULT_OPTS = dict(
    io_bufs=4,       # buffers of PAIR-sized (2*NT) load tiles
    q_bufs=6,
    qp_bufs=3,
    fp_bufs=3,
    sc_bufs=2,
    o_bufs=4,
    mega_bufs=4,
    small_bufs=8,
    gp_loads=True,   # issue tt/ft loads on SWDGE (gpsimd); tn on HWDGE
    pipe=5,          # stage_B(k) emitted after stage_A(k+pipe) (0 = no pipe)
    col_tiled=False,  # diag matmuls as 4 concurrent 32-col strips (measured slower)
    osb_eng="mix",   # engine for the fp->osb scale-copy: act/dve/mix/mix3
    noscc=False,     # skip the scores PSUM->SBUF copy; exp reads PSUM directly
    r_group=2,       # how many type-projections accumulate in PSUM at once
    mega_split=True,  # build the diag mega per block instead of one DVE op
)


def build_kernel_body_pt(ctx, tc, n_nodes, aps, opts=None, time_reps=None):
    o = dict(DEFAULT_OPTS, **(opts or {}))
    nc = tc.nc
    temb_n, temb_t, feat_t, w1t, w2t, mcol, eye, out = aps

    const = ctx.enter_context(tc.tile_pool(name="const", bufs=1))
    tio = ctx.enter_context(tc.tile_pool(name="tio", bufs=o["io_bufs"]))
    ttio = ctx.enter_context(tc.tile_pool(name="ttio", bufs=o["io_bufs"]))
    ftio = ctx.enter_context(tc.tile_pool(name="ftio", bufs=o["io_bufs"]))
    qpool = ctx.enter_context(tc.tile_pool(name="qpool", bufs=o["q_bufs"]))
    small = ctx.enter_context(tc.tile_pool(name="small", bufs=o["small_bufs"]))
    scpool = ctx.enter_context(tc.tile_pool(name="scpool", bufs=4))
    dpool = ctx.enter_context(tc.tile_pool(name="dpool", bufs=o["mega_bufs"]))
    opool = ctx.enter_context(tc.tile_pool(name="opool", bufs=o["o_bufs"]))
    qpsum = ctx.enter_context(tc.tile_pool(name="qpsum", bufs=o["qp_bufs"], space="PSUM"))
    spsum = ctx.enter_context(tc.tile_pool(name="spsum", bufs=o["sc_bufs"], space="PSUM"))
    fpsum = ctx.enter_context(tc.tile_pool(name="fpsum", bufs=o["fp_bufs"], space="PSUM"))

    w1sb = const.tile([128, 2, D2], BF16)
    w2sb = const.tile([128, 2, D2], FP8)
    msb = const.tile([128, 1], BF16)
    eyesb = const.tile([128, 128], F16)
    for c in range(2):
        nc.sync.dma_start(out=w1sb[:, c, :], in_=w1t[c])
        nc.sync.dma_start(out=w2sb[:, c, :], in_=w2t[c])
    nc.sync.dma_start(out=msb[:], in_=mcol[:])
    nc.sync.dma_start(out=eyesb[:], in_=eye[:])

    load_eng = nc.gpsimd if o["gp_loads"] else nc.sync
    PAIR = 2 * NT
    DR = mybir.MatmulPerfMode.DoubleRow

    rep_cm = tc.For_i(0, time_reps, 1) if time_reps else None
    if rep_cm is not None:
        ctx.enter_context(rep_cm)

    def stage_A(tn2, tt2, ft2, h, t0, nt):
        """Projection + scores for one NT-tile (half h of its pair)."""
        blocks = _sub_blocks(nt)
        n0 = h * NT
        scores = spsum.tile([128, 4 * R], F32, tag="sc")
        if o["r_group"] == 4:
            r_groups = [(0, 1, 2, 3)]
        elif o["r_group"] == 2:
            r_groups = [(0, 1), (2, 3)]
        else:
            r_groups = [(0,), (1,), (2,), (3,)]
        for rs in r_groups:
            qps = {}
            for r in rs:
                qps[r] = qpsum.tile([128, NT], F32, tag="qp", name="qp")
            for r in rs:
                nc.tensor.matmul(qps[r][:, 0:nt], w2sb[:, :, :],
                                 tt2[:, r, :, n0 : n0 + nt],
                                 start=True, stop=False, perf_mode=DR)
            for wi, c in enumerate([0, 1]):
                for r in rs:
                    nc.tensor.matmul(qps[r][:, 0:nt], w1sb[:, c, :],
                                     ft2[:, c, n0 : n0 + nt],
                                     start=False, stop=(wi == 1))
            for r in rs:
                q = qpool.tile([128, NT], BF16, tag="q")
                nc.scalar.activation(q[:, 0:nt], qps[r][:, 0:nt], ACTF.Tanh)
                # node n of this tile = 4*p + a (p-major layout): block a's
                # q columns are the stride-4 slice starting at a
                qv = q[:, 0:nt].rearrange("p (j a) -> p a j", a=4)
                for a, ns in blocks:
                    nc.tensor.matmul(
                        scores[0:ns, a * R + r : a * R + r + 1],
                        qv[:, a, 0:ns],
                        msb[:, 0:1],
                        start=True, stop=True,
                    )
        p = blocks[0][1]
        if o["noscc"]:
            return scores
        scc = scpool.tile([128, 4 * R], F32, tag="scc")
        nc.vector.tensor_copy(scc[0:p, 0 : R * len(blocks)],
                              scores[0:p, 0 : R * len(blocks)])
        return scc

    def stage_B(tn2, scc, h, t0, nt):
        """Softmax + beta-weighted output + store for one NT-tile."""
        blocks = _sub_blocks(nt)
        na = len(blocks)
        p = blocks[0][1]
        nr = R * na

        expo = small.tile([128, 4 * R], F16, tag="expo")
        nc.scalar.activation(expo[0:p, 0:nr], scc[0:p, 0:nr], ACTF.Exp)
        sume = small.tile([128, R], F32, tag="sume")
        nc.vector.tensor_reduce(
            sume[0:p, 0:na].unsqueeze(2),
            expo[0:p, 0:nr].rearrange("p (a r) -> p a r", a=na),
            AX.X, ALU.add,
        )
        inv = small.tile([128, R], F32, tag="inv")
        nc.vector.reciprocal(inv[0:p, 0:na], sume[0:p, 0:na])

        # All diag(exp) stationaries for this tile via broadcast DVE ops:
        # mega[p, t, c] = eye[p, c] * expo[p, t]
        mega = dpool.tile([128, 4 * R, 128], F16, tag="mega")
        if o["mega_split"]:
            for a, ns in blocks:
                eye_b = eyesb[0:ns, :].unsqueeze(1).broadcast_to([ns, R, 128])
                expo_b = (expo[0:ns, a * R : (a + 1) * R].unsqueeze(2)
                          .broadcast_to([ns, R, 128]))
                nc.vector.tensor_tensor(mega[0:ns, a * R : (a + 1) * R, :],
                                        eye_b, expo_b, ALU.mult)
        else:
            eye_b = eyesb[0:p, :].unsqueeze(1).broadcast_to([p, nr, 128])
            expo_b = expo[0:p, 0:nr].unsqueeze(2).broadcast_to([p, nr, 128])
            nc.vector.tensor_tensor(mega[0:p, 0:nr, :], eye_b, expo_b, ALU.mult)

        osb = opool.tile([128, 4, D], F16, tag="osb")
        for a, ns in blocks:
            fp = fpsum.tile([128, D], F32, tag="fp")
            if o["col_tiled"]:
                strips = [(j, min(32, ns - 32 * j)) for j in range((ns + 31) // 32)]
                for r in range(R):
                    for j, sj in strips:
                        nc.tensor.matmul(
                            fp[32 * j : 32 * j + sj, :],
                            mega[0:ns, a * R + r, 32 * j : 32 * j + sj],
                            tn2[0:ns, 4 * h + a, r, :],
                            start=(r == 0), stop=(r == R - 1),
                            tile_position=(0, 32 * j),
                            skip_group_check=True,
                        )
            else:
                for r in range(R):
                    nc.tensor.matmul(fp[0:ns, :], mega[0:ns, a * R + r, 0:ns],
                                     tn2[0:ns, 4 * h + a, r, :],
                                     start=(r == 0), stop=(r == R - 1))
            mode = o["osb_eng"]
            if mode == "mix3":
                eng = ("act", "act", "dve", "pool")[a % 4]
            elif mode == "mix":
                eng = "act" if a % 2 == 0 else "dve"
            else:
                eng = mode
            if eng == "act":
                nc.scalar.activation(osb[0:ns, a, :], fp[0:ns, :], ACTF.Copy,
                                     scale=inv[0:ns, a : a + 1])
            elif eng == "pool":
                nc.gpsimd.tensor_scalar_mul(osb[0:ns, a, :], fp[0:ns, :],
                                            inv[0:ns, a : a + 1])
            else:
                nc.vector.tensor_scalar_mul(osb[0:ns, a, :], fp[0:ns, :],
                                            inv[0:ns, a : a + 1])
        nc.sync.dma_start(
            out=out[t0 : t0 + nt].rearrange("(p a) d -> p a d", a=na),
            in_=osb[0:p, 0:na, :],
        )

    from collections import deque
    pend = deque()
    for p0 in range(0, n_nodes, PAIR):
        bnt = min(PAIR, n_nodes - p0)
        bp = min(PB, bnt)
        bna = (bnt + PB - 1) // PB
        tn2 = tio.tile([128, 8, R, D], F16, tag="tn")
        tt2 = ttio.tile([128, R, 2, PAIR], FP8, tag="tt")
        ft2 = ftio.tile([128, 2, PAIR], BF16, tag="ft")
        if bnt == PAIR and p0 > 0:
            for h in range(2):
                load_eng.dma_start(
                    out=tn2[:, 4 * h : 4 * h + 4, :, :],
                    in_=temb_n[p0 + h * NT : p0 + h * NT + NT].rearrange(
                        "(p a) r d -> p a r d", a=4
                    ),
                )
            load_eng.dma_start(
                out=tt2[:, :, :, 0:bnt],
                in_=temb_t[:, :, :, p0 : p0 + bnt].rearrange("r c p n -> p r c n"),
            )
            load_eng.dma_start(
                out=ft2[:, :, 0:bnt],
                in_=feat_t[:, :, p0 : p0 + bnt].rearrange("c p n -> p c n"),
            )
        else:
            # first pair and the ragged tail: per-half loads, so the first
            # projection can start after half the bytes have landed
            for h in range(2):
                t0 = p0 + h * NT
                if t0 >= n_nodes:
                    break
                nt = min(NT, n_nodes - t0)
                load_eng.dma_start(
                    out=ft2[:, :, h * NT : h * NT + nt],
                    in_=feat_t[:, :, t0 : t0 + nt].rearrange("c p n -> p c n"),
                )
                load_eng.dma_start(
                    out=tt2[:, :, :, h * NT : h * NT + nt],
                    in_=temb_t[:, :, :, t0 : t0 + nt].rearrange("r c p n -> p r c n"),
                )
                hna = min(4, nt)
                hp = (nt + 3) // 4
                load_eng.dma_start(
                    out=tn2[0:hp, 4 * h : 4 * h + hna, :, :],
                    in_=temb_n[t0 : t0 + nt].rearrange("(p a) r d -> p a r d", a=hna),
                )
        for h in range(2):
            t0 = p0 + h * NT
            if t0 >= n_nodes:
                break
            nt = min(NT, n_nodes - t0)
            scc = stage_A(tn2, tt2, ft2, h, t0, nt)
            pend.append((tn2, scc, h, t0, nt))
            if len(pend) > int(o["pipe"]):
                stage_B(*pend.popleft())
    while pend:
        stage_B(*pend.popleft())


def build_program_pt(n_nodes=N_NODES, num_devices=BS, opts=None, time_reps=None):
    nc = bacc.Bacc(
        "TRN2", target_bir_lowering=False, debug=False, num_devices=num_devices
    )
    temb_n = nc.dram_tensor("temb_n", [n_nodes, R, D], F16, kind="ExternalInput").ap()
    temb_t = nc.dram_tensor(
        "temb_t", [R, 2, 128, n_nodes], FP8, kind="ExternalInput"
    ).ap()
    feat_t = nc.dram_tensor("feat_t", [2, 128, n_nodes], BF16, kind="ExternalInput").ap()
    w1t = nc.dram_tensor("w1t", [2, 128, D2], BF16, kind="ExternalInput").ap()
    w2t = nc.dram_tensor("w2t", [2, 128, D2], FP8, kind="ExternalInput").ap()
    mcol = nc.dram_tensor("mcol", [D2, 1], BF16, kind="ExternalInput").ap()
    eye = nc.dram_tensor("eye", [128, 128], F16, kind="ExternalInput").ap()
    out = nc.dram_tensor("out", [n_nodes, D], F16, kind="ExternalOutput").ap()

    with tile.TileContext(nc) as tc, ExitStack() as ctx:
        build_kernel_body_pt(
            ctx, tc, n_nodes,
            (temb_n, temb_t, feat_t, w1t, w2t, mcol, eye, out), opts=opts,
            time_reps=time_reps,
        )
    nc.compile()
    return nc


def make_pt_inputs(feature, temb):
    """Host-side cast + transpose.
    feature: (bs, N, D) f32; temb: (bs, N, R, D) f32."""
    bf = ml_dtypes.bfloat16
    bs, n = feature.shape[0], feature.shape[1]
    temb_n = temb.astype(np.float16)  # (bs, N, R, D)
    temb_t = np.ascontiguousarray(
        temb.astype(ml_dtypes.float8_e4m3)
        .reshape(bs, n, R, 2, 128).transpose(0, 2, 3, 4, 1)
    )  # (bs, R, 2, 128, N)
    feat_t = np.ascontiguousarray(
        feature.astype(bf).reshape(bs, n, 2, 128).transpose(0, 2, 3, 1)
    )  # (bs, 2, 128, N)
    return temb_n, temb_t, feat_t


def make_const_inputs(w1, w2, m):
    bf = ml_dtypes.bfloat16
    w1t = np.ascontiguousarray(w1.T.astype(bf)).reshape(2, 128, D2)
    w2t = np.ascontiguousarray(w2.T.astype(ml_dtypes.float8_e4m3)).reshape(2, 128, D2)
    mcol = np.ascontiguousarray(m.reshape(D2, 1).astype(bf))
    eye = np.eye(128, dtype=np.float16)
    return w1t, w2t, mcol, eye


_cached_nc = None
_cached_opts = None


def kernel(feature, type_aware_emb, w1, w2, m, _trace=False, _tmpdir=None,
           _opts=None):
    from concourse.bass_utils import run_bass_kernel_spmd

    global _cached_nc, _cached_opts
    if _cached_nc is None or _opts != _cached_opts:
        _cached_nc = build_program_pt(opts=_opts)
        _cached_opts = _opts
    nc = _cached_nc

    w1t, w2t, mcol, eye = make_const_inputs(
        np.asarray(w1, np.float32), np.asarray(w2, np.float32),
        np.asarray(m, np.float32),
    )
    feature = np.asarray(feature, np.float32)
    temb = np.asarray(type_aware_emb, np.float32)
    temb_n, temb_t, feat_t = make_pt_inputs(feature, temb)
    in_maps = [
        {
            "temb_n": temb_n[i],
            "temb_t": temb_t[i],
            "feat_t": feat_t[i],
            "w1t": w1t,
            "w2t": w2t,
            "mcol": mcol,
            "eye": eye,
        }
        for i in range(BS)
    ]
    res = run_bass_kernel_spmd(
        nc, in_maps, list(range(BS)), trace=_trace, tmpdir=_tmpdir
    )
    out = np.stack([np.asarray(res.results[i]["out"]) for i in range(BS)])
    if _trace:
        kernel.last_result = res
    return out.reshape(BS, N_NODES, 1, D).astype(np.float32)


# revision 27
# speedup vs baseline: 1.0106x; 1.0106x over previous
"""Bass/Tile TRN2 kernel for nn_BatchAdditiveAttention.

Math (per batch, per node n):
    f_proj      = feature @ w1.T                        # (n, 128)
    t_proj[r]   = temb[:, r] @ w2.T                     # (n, 4, 128)
    q[r]        = tanh(f_proj + t_proj[r])              # (n, 4, 128)
    score[r]    = q[r] @ m                              # (n, 4)
    beta        = softmax_r(score)                      # (n, 4)
    out         = sum_r beta[r] * temb[:, r]            # (n, 256)

Sharding: data-parallel over bs=8, one batch per NeuronCore.

v3 layout/engine strategy (evolved from the v2 diag-stationary kernel;
v2 measured 368 us, v3 measures ~270-280 us on hw):
  - temb_t [R, 2, 128, N] fp8 + feat_t [2, 128, N] bf16 feed the
    projection matmuls (d on partitions, no on-chip transposes);
    the w2 projection runs as one fp8 DoubleRow matmul per type.
  - temb_n [N, R, D] fp16 is the natural-layout operand for the
    beta-weighted output reduction via diag(exp) stationaries.
    p-major node labeling (node = 4*p + a) makes both the temb_n load
    (8 KB contiguous per partition) and the output store (2 KB) long
    DMA rows; block a's q columns are the stride-4 slice a::4.
  - The 16 diag(exp) stationaries per 512-node tile are built in four
    per-block DVE tensor_tensor ops using double-broadcast APs
    (eye[p,c] x expo[p,t] -> mega[p,t,c]), replacing v2's 16 separate
    [128,128] tensor_scalar ops that made the DVE the 100%-busy
    bottleneck engine (DVE busy dropped 100% -> ~40%).
  - softmax: one batched ACT exp per tile ([128,16] -> fp16), one
    segmented DVE reduce for the per-block sums, one DVE reciprocal.
  - The final 1/sum scaling rides the PSUM->SBUF copy, alternating
    ACT Copy-with-scale / DVE tensor_scalar per block to balance the
    two engines (both sit at ~75% busy).
  - Projections process types in pairs (r_group=2) so the two PSUM
    accumulator banks recycle earlier; with qp=3/fp=3/sc=2 the eight
    PSUM banks sustain a 5-tile software pipeline (pipe=5).

Steady state: PE ~76% busy (projection + score + diag matmuls, LDW
re-emission per matmul is the main overhead), ACT ~75% (tanh + exp +
half the output copies), DMA ~235 us active vs a ~230 us floor for the
82 MB/core of traffic. Engine floors are balanced within ~15%.

Softmax skips the max-subtraction: scores are bounded (|score| <=
||m||_1 <= 11.4 in theory, ~2 in practice), well inside fp16 exp range.

Measured dead ends (do not revisit without new evidence): col-tiled
diag matmuls via tile_position (4x32-col strips, slower end-to-end);
m-stationary score matmuls (softmax then needs a partition-compacting
transpose, and strided-partition APs are rejected by the BIR verifier);
diagonal scatter-DMA into a pre-zeroed tile (illegal partition step);
walrus --enable-ldw-opt=true (rejects InstLdweights in this IR);
building diags on GpSimd (Pool cannot read PSUM, and tensor ops there
are ~2.5x DVE cost).
"""

import os
from contextlib import ExitStack

import numpy as np
import ml_dtypes

import concourse.bass as bass
import concourse.tile as tile
from concourse import bacc, mybir

BS = 8
N_NODES = 20000
D = 256
R = 4
D2 = 128
NT = 512  # nodes per tile
PB = 128  # nodes per sub-tile (partition block)

BF16 = mybir.dt.bfloat16
FP8 = mybir.dt.float8e4
F16 = mybir.dt.float16
F32 = mybir.dt.float32
AX = mybir.AxisListType
ALU = mybir.AluOpType
ACTF = mybir.ActivationFunctionType


def _sub_blocks(nt):
    """Split a node-tile of nt nodes into partition blocks of <=128.

    p-major node labeling: node n of the tile lives at partition n//4,
    block a = n%4, so block a holds ns_a = ceil((nt-a)/4) nodes.
    """
    return [(a, (nt - a + 3) // 4) for a in range(min(4, nt))]


DEFAULT_OPTS = dict(
    io_bufs=4,       # buffers of PAIR-sized (2*NT) load tiles
    q_bufs=6,
    qp_bufs=3,
    fp_bufs=3,
    sc_bufs=2,
    o_bufs=4,
    mega_bufs=4,
    small_bufs=8,
    gp_loads=True,   # issue tt/ft loads on SWDGE (gpsimd); tn on HWDGE
    pipe=5,          # stage_B(k) emitted after stage_A(k+pipe) (0 = no pipe)
    col_tiled=False,  # diag matmuls as 4 concurrent 32-col strips (measured slower)
    osb_eng="mix",   # engine for the fp->osb scale-copy: act/dve/mix/mix3
    noscc=False,     # skip the scores PSUM->SBUF copy; exp reads PSUM directly
    r_group=2,       # how many type-projections accumulate in PSUM at once
    mega_split=True,  # build the diag mega per block instead of one DVE op
)


def build_kernel_body_pt(ctx, tc, n_nodes, aps, opts=None, time_reps=None):
    o = dict(DEFAULT_OPTS, **(opts or {}))
    nc = tc.nc
    temb_n, temb_t, feat_t, w1t, w2t, mcol, eye, out = aps

    const = ctx.enter_context(tc.tile_pool(name="const", bufs=1))
    tio = ctx.enter_context(tc.tile_pool(name="tio", bufs=o["io_bufs"]))
    ttio = ctx.enter_context(tc.tile_pool(name="ttio", bufs=o["io_bufs"]))
    ftio = ctx.enter_context(tc.tile_pool(name="ftio", bufs=o["io_bufs"]))
    qpool = ctx.enter_context(tc.tile_pool(name="qpool", bufs=o["q_bufs"]))
    small = ctx.enter_context(tc.tile_pool(name="small", bufs=o["small_bufs"]))
    scpool = ctx.enter_context(tc.tile_pool(name="scpool", bufs=4))
    dpool = ctx.enter_context(tc.tile_pool(name="dpool", bufs=o["mega_bufs"]))
    opool = ctx.enter_context(tc.tile_pool(name="opool", bufs=o["o_bufs"]))
    qpsum = ctx.enter_context(tc.tile_pool(name="qpsum", bufs=o["qp_bufs"], space="PSUM"))
    spsum = ctx.enter_context(tc.tile_pool(name="spsum", bufs=o["sc_bufs"], space="PSUM"))
    fpsum = ctx.enter_context(tc.tile_pool(name="fpsum", bufs=o["fp_bufs"], space="PSUM"))

    w1sb = const.tile([128, 2, D2], BF16)
    w2sb = const.tile([128, 2, D2], FP8)
    msb = const.tile([128, 1], BF16)
    eyesb = const.tile([128, 128], F16)
    for c in range(2):
        nc.sync.dma_start(out=w1sb[:, c, :], in_=w1t[c])
        nc.sync.dma_start(out=w2sb[:, c, :], in_=w2t[c])
    nc.sync.dma_start(out=msb[:], in_=mcol[:])
    nc.sync.dma_start(out=eyesb[:], in_=eye[:])

    load_eng = nc.gpsimd if o["gp_loads"] else nc.sync
    PAIR = 2 * NT
    DR = mybir.MatmulPerfMode.DoubleRow

    rep_cm = tc.For_i(0, time_reps, 1) if time_reps else None
    if rep_cm is not None:
        ctx.enter_context(rep_cm)

    def stage_A(tn2, tt2, ft2, h, t0, nt):
        """Projection + scores for one NT-tile (half h of its pair)."""
        blocks = _sub_blocks(nt)
        n0 = h * NT
        scores = spsum.tile([128, 4 * R], F32, tag="sc")
        if o["r_group"] == 4:
            r_groups = [(0, 1, 2, 3)]
        elif o["r_group"] == 2:
            r_groups = [(0, 1), (2, 3)]
        else:
            r_groups = [(0,), (1,), (2,), (3,)]
        for rs in r_groups:
            qps = {}
            for r in rs:
                qps[r] = qpsum.tile([128, NT], F32, tag="qp", name="qp")
            for r in rs:
                nc.tensor.matmul(qps[r][:, 0:nt], w2sb[:, :, :],
                                 tt2[:, r, :, n0 : n0 + nt],
                                 start=True, stop=False, perf_mode=DR)
            for wi, c in enumerate([0, 1]):
                for r in rs:
                    nc.tensor.matmul(qps[r][:, 0:nt], w1sb[:, c, :],
                                     ft2[:, c, n0 : n0 + nt],
                                     start=False, stop=(wi == 1))
            for r in rs:
                q = qpool.tile([128, NT], BF16, tag="q")
                nc.scalar.activation(q[:, 0:nt], qps[r][:, 0:nt], ACTF.Tanh)
                # node n of this tile = 4*p + a (p-major layout): block a's
                # q columns are the stride-4 slice starting at a
                qv = q[:, 0:nt].rearrange("p (j a) -> p a j", a=4)
                for a, ns in blocks:
                    nc.tensor.matmul(
                        scores[0:ns, a * R + r : a * R + r + 1],
                        qv[:, a, 0:ns],
                        msb[:, 0:1],
                        start=True, stop=True,
                    )
        p = blocks[0][1]
        if o["noscc"]:
            return scores
        scc = scpool.tile([128, 4 * R], F32, tag="scc")
        nc.vector.tensor_copy(scc[0:p, 0 : R * len(blocks)],
                              scores[0:p, 0 : R * len(blocks)])
        return scc

    def stage_B(tn2, scc, h, t0, nt):
        """Softmax + beta-weighted output + store for one NT-tile."""
        blocks = _sub_blocks(nt)
        na = len(blocks)
        p = blocks[0][1]
        nr = R * na

        expo = small.tile([128, 4 * R], F16, tag="expo")
        nc.scalar.activation(expo[0:p, 0:nr], scc[0:p, 0:nr], ACTF.Exp)
        sume = small.tile([128, R], F32, tag="sume")
        nc.vector.tensor_reduce(
            sume[0:p, 0:na].unsqueeze(2),
            expo[0:p, 0:nr].rearrange("p (a r) -> p a r", a=na),
            AX.X, ALU.add,
        )
        inv = small.tile([128, R], F32, tag="inv")
        nc.vector.reciprocal(inv[0:p, 0:na], sume[0:p, 0:na])

        # All diag(exp) stationaries for this tile via broadcast DVE ops:
        # mega[p, t, c] = eye[p, c] * expo[p, t]
        mega = dpool.tile([128, 4 * R, 128], F16, tag="mega")
        if o["mega_split"]:
            for a, ns in blocks:
                eye_b = eyesb[0:ns, :].unsqueeze(1).broadcast_to([ns, R, 128])
                expo_b = (expo[0:ns, a * R : (a + 1) * R].unsqueeze(2)
                          .broadcast_to([ns, R, 128]))
                nc.vector.tensor_tensor(mega[0:ns, a * R : (a + 1) * R, :],
                                        eye_b, expo_b, ALU.mult)
        else:
            eye_b = eyesb[0:p, :].unsqueeze(1).broadcast_to([p, nr, 128])
            expo_b = expo[0:p, 0:nr].unsqueeze(2).broadcast_to([p, nr, 128])
            nc.vector.tensor_tensor(mega[0:p, 0:nr, :], eye_b, expo_b, ALU.mult)

        osb = opool.tile([128, 4, D], F16, tag="osb")
        for a, ns in blocks:
            fp = fpsum.tile([128, D], F32, tag="fp")
            if o["col_tiled"]:
                strips = [(j, min(32, ns - 32 * j)) for j in range((ns + 31) // 32)]
                for r in range(R):
                    for j, sj in strips:
                        nc.tensor.matmul(
                            fp[32 * j : 32 * j + sj, :],
                            mega[0:ns, a * R + r, 32 * j : 32 * j + sj],
                            tn2[0:ns, 4 * h + a, r, :],
                            start=(r == 0), stop=(r == R - 1),
                            tile_position=(0, 32 * j),
                            skip_group_check=True,
                        )
            else:
                for r in range(R):
                    nc.tensor.matmul(fp[0:ns, :], mega[0:ns, a * R + r, 0:ns],
                                     tn2[0:ns, 4 * h + a, r, :],
                                     start=(r == 0), stop=(r == R - 1))
            mode = o["osb_eng"]
            if mode == "mix3":
                eng = ("act", "act", "dve", "pool")[a % 4]
            elif mode == "mix":
                eng = "act" if a % 2 == 0 else "dve"
            else:
                eng = mode
            if eng == "act":
                nc.scalar.activation(osb[0:ns, a, :], fp[0:ns, :], ACTF.Copy,
                                     scale=inv[0:ns, a : a + 1])
            elif eng == "pool":
                nc.gpsimd.tensor_scalar_mul(osb[0:ns, a, :], fp[0:ns, :],
                                            inv[0:ns, a : a + 1])
            else:
                nc.vector.tensor_scalar_mul(osb[0:ns, a, :], fp[0:ns, :],
                                            inv[0:ns, a : a + 1])
        nc.sync.dma_start(
            out=out[t0 : t0 + nt].rearrange("(p a) d -> p a d", a=na),
            in_=osb[0:p, 0:na, :],
        )

    from collections import deque
    pend = deque()
    for p0 in range(0, n_nodes, PAIR):
        bnt = min(PAIR, n_nodes - p0)
        bp = min(PB, bnt)
        bna = (bnt + PB - 1) // PB
        tn2 = tio.tile([128, 8, R, D], F16, tag="tn")
        tt2 = ttio.tile([128, R, 2, PAIR], FP8, tag="tt")
        ft2 = ftio.tile([128, 2, PAIR], BF16, tag="ft")
        if bnt == PAIR and p0 > 0:
            for h in range(2):
                nc.sync.dma_start(
                    out=tn2[:, 4 * h : 4 * h + 4, :, :],
                    in_=temb_n[p0 + h * NT : p0 + h * NT + NT].rearrange(
                        "(p a) r d -> p a r d", a=4
                    ),
                )
            load_eng.dma_start(
                out=tt2[:, :, :, 0:bnt],
                in_=temb_t[:, :, :, p0 : p0 + bnt].rearrange("r c p n -> p r c n"),
            )
            load_eng.dma_start(
                out=ft2[:, :, 0:bnt],
                in_=feat_t[:, :, p0 : p0 + bnt].rearrange("c p n -> p c n"),
            )
        else:
            # first pair and the ragged tail: per-half loads, so the first
            # projection can start after half the bytes have landed
            for h in range(2):
                t0 = p0 + h * NT
                if t0 >= n_nodes:
                    break
                nt = min(NT, n_nodes - t0)
                load_eng.dma_start(
                    out=ft2[:, :, h * NT : h * NT + nt],
                    in_=feat_t[:, :, t0 : t0 + nt].rearrange("c p n -> p c n"),
                )
                load_eng.dma_start(
                    out=tt2[:, :, :, h * NT : h * NT + nt],
                    in_=temb_t[:, :, :, t0 : t0 + nt].rearrange("r c p n -> p r c n"),
                )
                hna = min(4, nt)
                hp = (nt + 3) // 4
                nc.sync.dma_start(
                    out=tn2[0:hp, 4 * h : 4 * h + hna, :, :],
                    in_=temb_n[t0 : t0 + nt].rearrange("(p a) r d -> p a r d", a=hna),
                )
        for h in range(2):
            t0 = p0 + h * NT
            if t0 >= n_nodes:
                break
            nt = min(NT, n_nodes - t0)
            scc = stage_A(tn2, tt2, ft2, h, t0, nt)
            pend.append((tn2, scc, h, t0, nt))
            if len(pend) > int(o["pipe"]):
                stage_B(*pend.popleft())
    while pend:
        stage_B(*pend.popleft())


def build_program_pt(n_nodes=N_NODES, num_devices=BS, opts=None, time_reps=None):
    nc = bacc.Bacc(
        "TRN2", target_bir_lowering=False, debug=False, num_devices=num_devices
    )
    temb_n = nc.dram_tensor("temb_n", [n_nodes, R, D], F16, kind="ExternalInput").ap()
    temb_t = nc.dram_tensor(
        "temb_t", [R, 2, 128, n_nodes], FP8, kind="ExternalInput"
    ).ap()
    feat_t = nc.dram_tensor("feat_t", [2, 128, n_nodes], BF16, kind="ExternalInput").ap()
    w1t = nc.dram_tensor("w1t", [2, 128, D2], BF16, kind="ExternalInput").ap()
    w2t = nc.dram_tensor("w2t", [2, 128, D2], FP8, kind="ExternalInput").ap()
    mcol = nc.dram_tensor("mcol", [D2, 1], BF16, kind="ExternalInput").ap()
    eye = nc.dram_tensor("eye", [128, 128], F16, kind="ExternalInput").ap()
    out = nc.dram_tensor("out", [n_nodes, D], F16, kind="ExternalOutput").ap()

    with tile.TileContext(nc) as tc, ExitStack() as ctx:
        build_kernel_body_pt(
            ctx, tc, n_nodes,
            (temb_n, temb_t, feat_t, w1t, w2t, mcol, eye, out), opts=opts,
            time_reps=time_reps,
        )
    nc.compile()
    return nc


def make_pt_inputs(feature, temb):
    """Host-side cast + transpose.
    feature: (bs, N, D) f32; temb: (bs, N, R, D) f32."""
    bf = ml_dtypes.bfloat16
    bs, n = feature.shape[0], feature.shape[1]
    temb_n = temb.astype(np.float16)  # (bs, N, R, D)
    temb_t = np.ascontiguousarray(
        temb.astype(ml_dtypes.float8_e4m3)
        .reshape(bs, n, R, 2, 128).transpose(0, 2, 3, 4, 1)
    )  # (bs, R, 2, 128, N)
    feat_t = np.ascontiguousarray(
        feature.astype(bf).reshape(bs, n, 2, 128).transpose(0, 2, 3, 1)
    )  # (bs, 2, 128, N)
    return temb_n, temb_t, feat_t


def make_const_inputs(w1, w2, m):
    bf = ml_dtypes.bfloat16
    w1t = np.ascontiguousarray(w1.T.astype(bf)).reshape(2, 128, D2)
    w2t = np.ascontiguousarray(w2.T.astype(ml_dtypes.float8_e4m3)).reshape(2, 128, D2)
    mcol = np.ascontiguousarray(m.reshape(D2, 1).astype(bf))
    eye = np.eye(128, dtype=np.float16)
    return w1t, w2t, mcol, eye


_cached_nc = None
_cached_opts = None


def kernel(feature, type_aware_emb, w1, w2, m, _trace=False, _tmpdir=None,
           _opts=None):
    from concourse.bass_utils import run_bass_kernel_spmd

    global _cached_nc, _cached_opts
    if _cached_nc is None or _opts != _cached_opts:
        _cached_nc = build_program_pt(opts=_opts)
        _cached_opts = _opts
    nc = _cached_nc

    w1t, w2t, mcol, eye = make_const_inputs(
        np.asarray(w1, np.float32), np.asarray(w2, np.float32),
        np.asarray(m, np.float32),
    )
    feature = np.asarray(feature, np.float32)
    temb = np.asarray(type_aware_emb, np.float32)
    temb_n, temb_t, feat_t = make_pt_inputs(feature, temb)
    in_maps = [
        {
            "temb_n": temb_n[i],
            "temb_t": temb_t[i],
            "feat_t": feat_t[i],
            "w1t": w1t,
            "w2t": w2t,
            "mcol": mcol,
            "eye": eye,
        }
        for i in range(BS)
    ]
    res = run_bass_kernel_spmd(
        nc, in_maps, list(range(BS)), trace=_trace, tmpdir=_tmpdir
    )
    out = np.stack([np.asarray(res.results[i]["out"]) for i in range(BS)])
    if _trace:
        kernel.last_result = res
    return out.reshape(BS, N_NODES, 1, D).astype(np.float32)
